# revision 18
# baseline (speedup 1.0000x reference)
"""Entmax multihead attention on 8 Trainium2 NeuronCores (Bass/Tile).

Sharding: core c <-> (batch b = c//2, head-group hg = c%2, 8 heads each).
Per core: project q/k/v for its 8 heads (q,k kept transposed [hd, n]),
scores via PE, entmax-bisect replaced by a numerically-equivalent
bracketed Illinois root-find (E=9 evals vs reference's 50 bisections;
tau* is provably in [-1, 0] after the row-max shift), attn written to
DRAM, attn transposed on PE for ctx = attn @ v, out-projection partials
ReduceScattered across the batch pair.
"""

import sys

sys.path.insert(0, "/opt/trn_rl_repo")

import os

import numpy as np

DEBUG_PHASES = os.environ.get("KERNEL_PHASES", "FULL")

B, N, D, H, HD = 4, 1024, 1024, 16, 64
HPG = 8          # heads per core
HDG = HPG * HD   # 512 head-dims per core
ALPHA = 1.3
CEXP = 1.0 / (ALPHA - 1.0)          # 10/3
SCL = (ALPHA - 1.0) / 8.0           # (alpha-1)/sqrt(hd) = 0.0375
E_ITERS = 6
NT = N // 128                        # 8 row tiles per score matrix

_NC_CACHE = None


def _build_nc():
    import concourse.bass as bass
    import concourse.mybir as mybir
    from concourse.tile import TileContext
    from concourse.vector_clock import ScopedClock
    from concourse.masks import make_identity
    from contextlib import ExitStack

    dt = mybir.dt.float32
    AF = mybir.ActivationFunctionType
    ALU = mybir.AluOpType
    AX = mybir.AxisListType

    class PatchedTileContext(TileContext):
        # This walrus build encodes at most ONE sem wait per instruction
        # ("Too many sync wait commands" in setupSyncWait). After
        # scheduling, hoist surplus waits onto same-engine NoOps inserted
        # immediately before the carrying instruction — identical blocking
        # semantics, encodable.
        MAXW = 1

        def schedule_and_allocate(self):
            ret = super().schedule_and_allocate()
            n_added = 0
            for fn in self.nc.m.functions:
                for bb in fn.blocks:
                    out = []
                    for inst in bb.instructions:
                        si = getattr(inst, "sync_info", None)
                        waits = list(si.on_wait) if si is not None and si.on_wait else []
                        if len(waits) > self.MAXW and inst.engine != mybir.EngineType.Unassigned:
                            si.on_wait = waits[-self.MAXW :]
                            extra = waits[: -self.MAXW]
                            for w in extra:
                                nop = mybir.InstNoOp(
                                    name=f"{inst.name}-wsplit-{n_added}",
                                    ins=[],
                                    outs=[],
                                )
                                nop.engine = inst.engine
                                nop.sync_info = mybir.SyncInfo(on_update=[], on_wait=[w])
                                self.nc.register_instruction(nop, overwrite=True)
                                out.append(nop)
                                n_added += 1
                        out.append(inst)
                    bb.instructions = out
            return ret

    nc = bass.Bass("TRN2", target_bir_lowering=False, debug=False, num_devices=8)

    xb = nc.dram_tensor("xb", [N, D], dt, kind="ExternalInput")
    wq = nc.dram_tensor("wq", [D, HDG], dt, kind="ExternalInput")
    wk = nc.dram_tensor("wk", [D, HDG], dt, kind="ExternalInput")
    wv = nc.dram_tensor("wv", [D, HDG], dt, kind="ExternalInput")
    wo = nc.dram_tensor("wo", [HDG, D], dt, kind="ExternalInput")
    bq = nc.dram_tensor("bq", [HDG], dt, kind="ExternalInput")
    bk = nc.dram_tensor("bk", [HDG], dt, kind="ExternalInput")
    bv = nc.dram_tensor("bv", [HDG], dt, kind="ExternalInput")
    attn_out = nc.dram_tensor("attn_out", [HPG, N, N], dt, kind="ExternalOutput")
    partial_out = nc.dram_tensor("partial_out", [N, D], dt, kind="ExternalOutput")

    with PatchedTileContext(nc) as tc, ExitStack() as ctx:
        const = ctx.enter_context(tc.tile_pool(name="const", bufs=1))
        ident = const.tile([128, 128], dt)
        make_identity(nc, ident[:])
        bq_sb = const.tile([128, 4], dt)
        nc.sync.dma_start(out=bq_sb[:], in_=bq.ap().rearrange("(g p) -> p g", p=128))
        bk_sb = const.tile([128, 4], dt)
        nc.sync.dma_start(out=bk_sb[:], in_=bk.ap().rearrange("(g p) -> p g", p=128))
        bv_sb = const.tile([128, 4], dt)
        nc.sync.dma_start(out=bv_sb[:], in_=bv.ap().rearrange("(g p) -> p g", p=128))
        eps_u = const.tile([128, 1], dt)
        nc.vector.memset(eps_u[:], 1e-35)
        eps_s = const.tile([128, 1], dt)
        nc.vector.memset(eps_s[:], 1e-30)

        persist = ctx.enter_context(tc.tile_pool(name="persist", bufs=1))
        qT = persist.tile([128, 4, N], dt)
        kT = persist.tile([128, 4, N], dt)
        v_sb = persist.tile([128, NT, HDG], dt)
        ctxT = persist.tile([128, 4, N], dt)

        # ---- Phase A: x transpose + q/k/v projections -------------------
        with (
            tc.tile_pool(name="projA", bufs=1) as pa,
            tc.tile_pool(name="psA", bufs=4, space="PSUM") as psA,
            tc.tile_pool(name="psT", bufs=2, space="PSUM") as psT,
        ):
            xt = pa.tile([128, NT, N], dt)
            for t in range(NT):
                xa = pa.tile([128, D], dt, tag="xa", bufs=2)
                nc.sync.dma_start(out=xa[:], in_=xb[t * 128 : (t + 1) * 128, :])
                for j in range(8):
                    pt = psT.tile([128, 128], dt)
                    nc.tensor.transpose(pt[:], xa[:, j * 128 : (j + 1) * 128], ident[:])
                    nc.vector.tensor_copy(xt[:, j, t * 128 : (t + 1) * 128], pt[:])
            wq_sb = pa.tile([128, 8, HDG], dt)
            nc.sync.dma_start(out=wq_sb[:], in_=wq.ap().rearrange("(kc p) n -> p kc n", p=128))
            wk_sb = pa.tile([128, 8, HDG], dt)
            nc.sync.dma_start(out=wk_sb[:], in_=wk.ap().rearrange("(kc p) n -> p kc n", p=128))
            wv_sb = pa.tile([128, 8, HDG], dt)
            nc.sync.dma_start(out=wv_sb[:], in_=wv.ap().rearrange("(kc p) n -> p kc n", p=128))

            for wt, bt, dst in ((wq_sb, bq_sb, qT), (wk_sb, bk_sb, kT)):
                for g in range(4):
                    for nch in range(2):
                        ps = psA.tile([128, 512], dt)
                        for kc in range(8):
                            nc.tensor.matmul(
                                ps[:],
                                wt[:, kc, g * 128 : (g + 1) * 128],
                                xt[:, kc, nch * 512 : (nch + 1) * 512],
                                start=(kc == 0),
                                stop=(kc == 7),
                            )
                        nc.vector.tensor_scalar(
                            dst[:, g, nch * 512 : (nch + 1) * 512], ps[:],
                            bt[:, g : g + 1], None, op0=ALU.add,
                        )
            for t in range(NT):
                ps = psA.tile([128, 512], dt)
                for kc in range(8):
                    nc.tensor.matmul(
                        ps[:],
                        xt[:, kc, t * 128 : (t + 1) * 128],
                        wv_sb[:, kc, :],
                        start=(kc == 0),
                        stop=(kc == 7),
                    )
                nc.vector.tensor_copy(v_sb[:, t, :], ps[:])

        # ---- Phase B: per-head scores + entmax + ctx --------------------
        n_heads = {"A": 0, "B1": 1, "B": HPG}.get(DEBUG_PHASES, HPG)
        with (
            tc.tile_pool(name="bigwork", bufs=2) as ypool,
            tc.tile_pool(name="upool", bufs=2) as upool,
            tc.tile_pool(name="lpool", bufs=2) as lpool,
            tc.tile_pool(name="ppool", bufs=2) as ppool,
            tc.tile_pool(name="atp", bufs=2) as atpool,
            tc.tile_pool(name="state", bufs=2) as spool,
            tc.tile_pool(name="ps_s", bufs=2, space="PSUM") as ps_s_pool,
            tc.tile_pool(name="ps_c", bufs=1, space="PSUM") as ps_c_pool,
            tc.tile_pool(name="ps_t2", bufs=2, space="PSUM") as ps_t2,
        ):
            def small_tt(op, a, b, tag):
                o = spool.tile([128, NT], dt, tag=tag)
                nc.vector.tensor_tensor(o[:], a[:], b[:], op=op)
                return o

            def small_ts(a, s1, s2, op0, op1, tag):
                o = spool.tile([128, NT], dt, tag=tag)
                if s2 is None:
                    nc.vector.tensor_scalar(o[:], a[:], s1, None, op0=op0)
                else:
                    nc.vector.tensor_scalar(o[:], a[:], s1, s2, op0=op0, op1=op1)
                return o

            for h in range(n_heads):
                po = (h % 2) * 64
                g = h // 2
                y = ypool.tile([128, NT, N], dt, tag="big", name="y")
                mxn = spool.tile([128, NT], dt, tag="mxn")
                for t in range(NT):
                    ps = ps_s_pool.tile([128, N], dt, tag="scores")
                    for mch in range(2):
                        nc.tensor.matmul(
                            ps[:, mch * 512 : (mch + 1) * 512],
                            qT[po : po + 64, g, t * 128 : (t + 1) * 128],
                            kT[po : po + 64, g, mch * 512 : (mch + 1) * 512],
                            start=True,
                            stop=True,
                            tile_position=(po, 0),
                        )
                    mx = spool.tile([128, 1], dt, tag="mx")
                    nc.vector.reduce_max(mx[:], ps[:, :], axis=AX.X)
                    nc.vector.tensor_scalar(mxn[:, t : t + 1], mx[:], -SCL, None, op0=ALU.mult)
                    nc.vector.tensor_scalar(
                        y[:, t, :], ps[:, :], SCL, mxn[:, t : t + 1],
                        op0=ALU.mult, op1=ALU.add,
                    )

                # Bracketed Illinois root-find for tau in [-1, 0].
                lo = spool.tile([128, NT], dt, tag="lo")
                nc.vector.memset(lo[:], -1.0)
                hi = spool.tile([128, NT], dt, tag="hi")
                nc.vector.memset(hi[:], 0.0)
                flo = spool.tile([128, NT], dt, tag="flo")
                nc.vector.memset(flo[:], 1.0)
                fhi = spool.tile([128, NT], dt, tag="fhi")
                nc.vector.memset(fhi[:], -1.0)
                pgt = spool.tile([128, NT], dt, tag="pgt")
                nc.vector.memset(pgt[:], 0.0)
                s_all = spool.tile([128, NT], dt, tag="s_all")

                def eval_s(tau):
                    # s_all[:, t] = sum_m relu(y[t] - tau_t)^CEXP
                    for tp in range(NT // 2):
                        u = upool.tile([128, 2, N], dt, tag="u")
                        for j in range(2):
                            ti = tp * 2 + j
                            nc.vector.tensor_scalar(
                                u[:, j, :], y[:, ti, :], tau[:, ti : ti + 1], 0.0,
                                op0=ALU.subtract, op1=ALU.max,
                            )
                        l = lpool.tile([128, 2, N], dt, tag="l")
                        nc.scalar.activation(l[:], u[:], AF.Ln, bias=eps_u[:, 0:1])
                        for j in range(2):
                            ti = tp * 2 + j
                            pj = ppool.tile([128, N], dt, tag="pscr")
                            nc.scalar.activation(
                                pj[:], l[:, j, :], AF.Exp, scale=CEXP,
                                accum_out=s_all[:, ti : ti + 1],
                            )
                            yield ti, pj

                for e in range(E_ITERS):
                    d = small_tt(ALU.subtract, flo, fhi, "d")
                    dm = small_ts(d, 1e-20, None, ALU.max, None, "dm")
                    r = spool.tile([128, NT], dt, tag="r")
                    nc.vector.reciprocal(r[:], dm[:])
                    w = small_tt(ALU.subtract, hi, lo, "w")
                    n1 = small_tt(ALU.mult, w, flo, "n1")
                    n2 = small_tt(ALU.mult, n1, r, "n2")
                    t0 = small_tt(ALU.add, lo, n2, "t0")
                    tcur = small_tt(ALU.min, t0, hi, "tcur")
                    for _ in eval_s(tcur):
                        pass
                    sl = spool.tile([128, NT], dt, tag="sl")
                    nc.scalar.activation(sl[:], s_all[:], AF.Ln, bias=eps_s[:, 0:1])
                    sf = spool.tile([128, NT], dt, tag="sf")
                    nc.scalar.activation(sf[:], sl[:], AF.Exp, scale=1.0 / CEXP)
                    f = small_ts(sf, -1.0, None, ALU.add, None, "f")
                    gt = small_ts(f, 0.0, None, ALU.is_gt, None, "gt")
                    # Illinois halving of the retained endpoint
                    a = small_ts(pgt, -0.5, 1.0, ALU.mult, ALU.add, "a")
                    fhic = small_tt(ALU.mult, fhi, a, "fhic")
                    dh = small_tt(ALU.subtract, fhic, f, "dh")
                    dh2 = small_tt(ALU.mult, gt, dh, "dh2")
                    fhi = small_tt(ALU.add, f, dh2, "fhi2")
                    bfac = small_ts(pgt, 0.5, 0.5, ALU.mult, ALU.add, "bfac")
                    floc = small_tt(ALU.mult, flo, bfac, "floc")
                    dl = small_tt(ALU.subtract, f, floc, "dl")
                    dl2 = small_tt(ALU.mult, gt, dl, "dl2")
                    flo = small_tt(ALU.add, floc, dl2, "flo2")
                    dtl = small_tt(ALU.subtract, tcur, lo, "dtl")
                    dtl2 = small_tt(ALU.mult, gt, dtl, "dtl2")
                    lo = small_tt(ALU.add, lo, dtl2, "lo2")
                    ngt = small_ts(gt, -1.0, 1.0, ALU.mult, ALU.add, "ngt")
                    dth = small_tt(ALU.subtract, tcur, hi, "dth")
                    dth2 = small_tt(ALU.mult, ngt, dth, "dth2")
                    hi = small_tt(ALU.add, hi, dth2, "hi2")
                    pgt = gt

                # Final pass at hi (matches reference's use of the upper end).
                attn = ypool.tile([128, NT, N], dt, tag="big", name="attn")
                for ti, pj in eval_s(hi):
                    sr = spool.tile([128, 1], dt, tag="sr")
                    nc.vector.tensor_scalar(sr[:], s_all[:, ti : ti + 1], 1e-12, None, op0=ALU.add)
                    rr = spool.tile([128, 1], dt, tag="rr")
                    nc.vector.reciprocal(rr[:], sr[:])
                    nc.vector.tensor_scalar(attn[:, ti, :], pj[:], rr[:], None, op0=ALU.mult)
                    nc.sync.dma_start(
                        out=attn_out[h, ti * 128 : (ti + 1) * 128, :], in_=attn[:, ti, :]
                    )

                # ctxT[h] = (attn @ v_h)^T via PE transposes of attn chunks
                ps_c = ps_c_pool.tile([128, N], dt, tag="ctx")
                for mch in range(NT):
                    at = atpool.tile([128, N], dt, tag="at")
                    for nt in range(NT):
                        pt = ps_t2.tile([128, 128], dt, tag="tr")
                        nc.tensor.transpose(
                            pt[:], attn[:, nt, mch * 128 : (mch + 1) * 128], ident[:]
                        )
                        nc.vector.tensor_copy(at[:, nt * 128 : (nt + 1) * 128], pt[:])
                    for nch in range(2):
                        nc.tensor.matmul(
                            ps_c[po : po + 64, nch * 512 : (nch + 1) * 512],
                            v_sb[:, mch, h * 64 : (h + 1) * 64],
                            at[:, nch * 512 : (nch + 1) * 512],
                            start=(mch == 0),
                            stop=(mch == NT - 1),
                            tile_position=(0, po),
                        )
                nc.vector.tensor_scalar(
                    ctxT[po : po + 64, g, :], ps_c[po : po + 64, :],
                    bv_sb[po : po + 64, g : g + 1], None, op0=ALU.add,
                )

        # ---- Phase C: out projection (pair-sum + bo happen on host) -----
        if DEBUG_PHASES in ("FULL", "C_NOCC"):
            opool = ctx.enter_context(tc.tile_pool(name="oproj", bufs=4))
            wopool = ctx.enter_context(tc.tile_pool(name="wop", bufs=1))
            ps_o_pool = ctx.enter_context(tc.tile_pool(name="ps_o", bufs=4, space="PSUM"))
            wo_sb = wopool.tile([128, 4, D], dt)
            nc.sync.dma_start(out=wo_sb[:], in_=wo.ap().rearrange("(kc p) d -> p kc d", p=128))
            for t in range(NT):
                for dch in range(2):
                    ps = ps_o_pool.tile([128, 512], dt)
                    for kc in range(4):
                        nc.tensor.matmul(
                            ps[:],
                            ctxT[:, kc, t * 128 : (t + 1) * 128],
                            wo_sb[:, kc, dch * 512 : (dch + 1) * 512],
                            start=(kc == 0),
                            stop=(kc == 3),
                        )
                    ot = opool.tile([128, 512], dt, tag="ot")
                    nc.vector.tensor_copy(ot[:], ps[:])
                    nc.sync.dma_start(
                        out=partial_out[t * 128 : (t + 1) * 128, dch * 512 : (dch + 1) * 512],
                        in_=ot[:],
                    )

    return nc


def _get_nc():
    global _NC_CACHE
    if _NC_CACHE is None:
        _NC_CACHE = _build_nc()
    return _NC_CACHE


def kernel(x, Wq, bq, Wk, bk, Wv, bv, Wo, bo):
    from concourse.bass_utils import run_bass_kernel_spmd

    x = np.asarray(x, dtype=np.float32)
    Wq, Wk, Wv, Wo = (np.asarray(a, dtype=np.float32) for a in (Wq, Wk, Wv, Wo))
    bq, bk, bv, bo = (np.asarray(a, dtype=np.float32) for a in (bq, bk, bv, bo))

    nc = _get_nc()
    in_maps = []
    for c in range(8):
        b, hg = divmod(c, 2)
        cs = slice(hg * HDG, (hg + 1) * HDG)
        in_maps.append(
            {
                "xb": np.ascontiguousarray(x[b]),
                "wq": np.ascontiguousarray(Wq[:, cs]),
                "wk": np.ascontiguousarray(Wk[:, cs]),
                "wv": np.ascontiguousarray(Wv[:, cs]),
                "wo": np.ascontiguousarray(Wo[cs, :]),
                "bq": np.ascontiguousarray(bq[cs]),
                "bk": np.ascontiguousarray(bk[cs]),
                "bv": np.ascontiguousarray(bv[cs]),
            }
        )
    res = run_bass_kernel_spmd(nc, in_maps, list(range(8))).results

    out = np.empty((B, N, D), np.float32)
    attn = np.empty((B, H, N, N), np.float32)
    for b in range(B):
        attn[b, :HPG] = res[2 * b]["attn_out"]
        attn[b, HPG:] = res[2 * b + 1]["attn_out"]
        out[b] = res[2 * b]["partial_out"] + res[2 * b + 1]["partial_out"]
    out += bo[None, None, :]
    return out, attn
